# revision 41
# baseline (speedup 1.0000x reference)
"""Multi-head attention (B=2, T=2048, D=1024, H=16, causal) on 8 TRN2 NeuronCores.

Sharding strategy (tensor-parallel over heads + token-parallel epilogue):
  - Each core c owns heads (2c, 2c+1)  -> a 128-wide slice of the QKV output dim.
  - QKV projections:  qT/kT/vT [128, B*T] computed from replicated x^T
    (feature-major) with host-pre-transposed weight slices.
  - Attention: flash-style streaming over 128-wide key blocks with
    transposed score tiles  S^T [k=128, q=512]; exp on ScalarE; causal
    staircase mask via a precomputed [128, 1024] 0/1 constant (DVE mult,
    diagonal tiles only); ctx^T and softmax denominators accumulate in PSUM
    (ones-column matmuls for the partition-dim sums).
  - AllToAll over the token dim redistributes ctx^T so core c holds
    ctx[tokens 512c:512(c+1), all 1024 cols], i.e. the minimal-traffic
    exchange (2 MB per core).
  - Output projection is token-sharded: each core computes its 512 output
    rows with the full (host-pre-transposed) Wo^T; host concatenates.

All matmuls run as float32r (1 cycle/row when the moving dim >= 256);
transposes stay float32 (exact).
"""

import numpy as np

import concourse.bacc as bacc
import concourse.bass as bass
import concourse.mybir as mybir
import concourse.tile as tile
from concourse import bass_utils
from concourse.bass import ts

D = 1024
H = 16
DK = D // H  # 64
NCORES = 8
HPC = H // NCORES  # heads per core = 2
DSL = HPC * DK  # per-core QKV output slice = 128
P = 128
QBLK = 512
KBLK = 128

F32 = mybir.dt.float32
F32R = mybir.dt.float32r


def build_nc(B=2, T=2048):
    """Build the SPMD Bass module (identical program on all 8 cores)."""
    NTOK = B * T
    TPC = NTOK // NCORES  # tokens per core for the output projection
    KO = D // P  # 8 contraction chunks for the projections
    NKB = T // KBLK  # key blocks per batch
    NQB = T // QBLK  # query blocks per batch
    TB = TPC // P  # 128-token sub-blocks in the output projection

    nc = bacc.Bacc("TRN2", target_bir_lowering=False, debug=False,
                   num_devices=NCORES)

    # ---- DRAM I/O ------------------------------------------------------
    xT_d = nc.dram_tensor("xT", [D, NTOK], F32R, kind="ExternalInput")
    wqT_d = nc.dram_tensor("wqT", [D, DSL], F32R, kind="ExternalInput")
    wkT_d = nc.dram_tensor("wkT", [D, DSL], F32R, kind="ExternalInput")
    wvT_d = nc.dram_tensor("wvT", [D, DSL], F32R, kind="ExternalInput")
    woT_d = nc.dram_tensor("woT", [D, D], F32R, kind="ExternalInput")
    bq_d = nc.dram_tensor("bq", [DSL, 1], F32, kind="ExternalInput")
    bk_d = nc.dram_tensor("bk", [DSL, 1], F32, kind="ExternalInput")
    bv_d = nc.dram_tensor("bv", [DSL, 1], F32, kind="ExternalInput")
    bo_d = nc.dram_tensor("bo", [D], F32, kind="ExternalInput")
    mask_d = nc.dram_tensor("mask", [P, 2 * QBLK - KBLK], F32, kind="ExternalInput")
    ident_d = nc.dram_tensor("ident", [P, P], F32R, kind="ExternalInput")
    ones_d = nc.dram_tensor("ones", [P, P], F32R, kind="ExternalInput")
    out_d = nc.dram_tensor("out", [TPC, D], F32, kind="ExternalOutput")

    with tile.TileContext(nc) as tc:
        with (
            tc.tile_pool(name="consts", bufs=1) as consts,
            tc.tile_pool(name="acts", bufs=1) as acts,
            tc.tile_pool(name="xin", bufs=2) as xin,
            tc.tile_pool(name="attn", bufs=3) as attn_pool,
            tc.tile_pool(name="small", bufs=1) as small,
            tc.tile_pool(name="outg", bufs=1) as outg,
            tc.tile_pool(name="outp", bufs=2) as outp,
            tc.tile_pool(name="psA", bufs=4, space="PSUM") as psA,
            tc.tile_pool(name="psC", bufs=1, space="PSUM") as psC,
            tc.tile_pool(name="psM", bufs=2, space="PSUM") as psM,
            tc.tile_pool(name="dram", bufs=1, space="DRAM") as dram,
        ):
            # ---- constants / weights in SBUF ----------------------------
            wq_sb = consts.tile([P, KO, DSL], F32R, tag="wq")
            wk_sb = consts.tile([P, KO, DSL], F32R, tag="wk")
            wv_sb = consts.tile([P, KO, DSL], F32R, tag="wv")
            wo_sb = consts.tile([P, KO, D], F32R, tag="wo")
            nc.sync.dma_start(wq_sb[:], wqT_d.ap().rearrange("(ko p) m -> p ko m", p=P))
            nc.sync.dma_start(wk_sb[:], wkT_d.ap().rearrange("(ko p) m -> p ko m", p=P))
            nc.sync.dma_start(wv_sb[:], wvT_d.ap().rearrange("(ko p) m -> p ko m", p=P))
            nc.sync.dma_start(wo_sb[:], woT_d.ap().rearrange("(ko p) m -> p ko m", p=P))
            bq_sb = consts.tile([P, 1], F32, tag="bq")
            bk_sb = consts.tile([P, 1], F32, tag="bk")
            bv_sb = consts.tile([P, 1], F32, tag="bv")
            nc.sync.dma_start(bq_sb[:], bq_d.ap())
            nc.sync.dma_start(bk_sb[:], bk_d.ap())
            nc.sync.dma_start(bv_sb[:], bv_d.ap())
            bo_sb = consts.tile([P, D], F32, tag="bo")
            nc.sync.dma_start(bo_sb[:], bo_d.ap()[None, :].to_broadcast((P, D)))
            mask_sb = consts.tile([P, 2 * QBLK - KBLK], F32, tag="mask")
            nc.sync.dma_start(mask_sb[:], mask_d.ap())
            ident_sb = consts.tile([P, P], F32R, tag="ident")
            nc.sync.dma_start(ident_sb[:], ident_d.ap())
            ones_sb = consts.tile([P, P], F32R, tag="ones")
            nc.sync.dma_start(ones_sb[:], ones_d.ap())

            # ---- phase 1: QKV projections (feature-major outputs) -------
            qT = acts.tile([P, NTOK], F32R, tag="qT")
            kT = acts.tile([P, NTOK], F32R, tag="kT")
            vT = acts.tile([P, NTOK], F32R, tag="vT")
            xT_r = xT_d.ap().rearrange("(ko p) t -> p ko t", p=P)
            n_ttile = NTOK // QBLK
            for i in range(n_ttile):
                xt = xin.tile([P, KO, QBLK], F32R, tag="xt")
                nc.sync.dma_start(xt[:], xT_r[:, :, ts(i, QBLK)])
                for w_sb, b_sb, dst in ((wq_sb, bq_sb, qT),
                                        (wk_sb, bk_sb, kT),
                                        (wv_sb, bv_sb, vT)):
                    ps = psA.tile([P, QBLK], F32, tag="pp")
                    for ko in range(KO):
                        nc.tensor.matmul(ps[:], w_sb[:, ko], xt[:, ko],
                                         start=(ko == 0), stop=(ko == KO - 1))
                    nc.scalar.activation(dst[:, ts(i, QBLK)], ps[:],
                                         mybir.ActivationFunctionType.Identity,
                                         bias=b_sb[:, 0:1])

            # ---- phase 1.5: v -> natural layout [ktok, d|1] -------------
            # per key-block: [v_h0 (64) | ones (1) | v_h1 (64) | ones (1)]
            # so the ctx matmul's 65th output row is the softmax denominator.
            DA = DK + 1  # 65
            v_nat = acts.tile([P, NTOK // P, 2 * DA], F32R, tag="v_nat")
            nc.sync.dma_start(v_nat[:, :, DK], ones_d.ap()[:, 0:NTOK // P])
            nc.sync.dma_start(v_nat[:, :, DA + DK], ones_d.ap()[:, 0:NTOK // P])
            for j in range(NTOK // P):
                ptf = psM.tile([P, QBLK], F32R, tag="misc", name="ptf")
                pt = ptf[:, :P]
                nc.tensor.transpose(pt[:], vT[:, ts(j, P)], ident_sb[:])
                nc.vector.tensor_copy(v_nat[:, j, 0:DK], pt[:, 0:DK])
                nc.vector.tensor_copy(v_nat[:, j, DA:DA + DK], pt[:, DK:P])

            # ---- phase 2: causal attention (2 heads, feature-major) -----
            a2a_in = dram.tile([NCORES, P, TPC], F32R, tag="a2a_in")
            a2a_out = dram.tile([NCORES, P, TPC], F32R, tag="a2a_out")
            for b in range(B):
                for qi in range(NQB):
                    q_sl = ts(b * T // QBLK + qi, QBLK)  # global token slice
                    nkb = (qi + 1) * (QBLK // KBLK)
                    C0 = psC.tile([P, QBLK], F32, tag="ctx0")
                    C1 = psC.tile([P, QBLK], F32, tag="ctx1")
                    for ki in range(nkb):
                        k_sl = ts(b * T // KBLK + ki, KBLK)
                        jj = b * NKB + ki
                        s0 = psA.tile([P, QBLK], F32, tag="pp")
                        s1 = psA.tile([P, QBLK], F32, tag="pp")
                        nc.tensor.matmul(s0[:], kT[0:DK, k_sl], qT[0:DK, q_sl],
                                         start=True, stop=True, tile_position=(0, 0))
                        nc.tensor.matmul(s1[:], kT[DK:P, k_sl], qT[DK:P, q_sl],
                                         start=True, stop=True, tile_position=(64, 0))
                        a0 = attn_pool.tile([P, QBLK], F32R, tag="a0")
                        a1 = attn_pool.tile([P, QBLK], F32R, tag="a1")
                        nc.scalar.activation(a0[:], s0[:],
                                             mybir.ActivationFunctionType.Exp)
                        nc.scalar.activation(a1[:], s1[:],
                                             mybir.ActivationFunctionType.Exp)
                        doff = ki * KBLK - qi * QBLK
                        if doff >= 0:  # diagonal tile: apply causal staircase
                            s = QBLK - KBLK - doff
                            m = mask_sb[:, s:s + QBLK]
                            nc.vector.tensor_mul(a0[:], a0[:], m)
                            nc.vector.tensor_mul(a1[:], a1[:], m)
                        st = (ki == 0)
                        sp = (ki == nkb - 1)
                        nc.tensor.matmul(C0[0:DA], v_nat[:, jj, 0:DA], a0[:],
                                         start=st, stop=sp)
                        nc.tensor.matmul(C1[0:DA], v_nat[:, jj, DA:2 * DA], a1[:],
                                         start=st, stop=sp)
                    # normalize: ctx^T * (1/sum) broadcast over partitions
                    rec = small.tile([P, 2 * QBLK], F32R, tag="rec")
                    with nc.allow_low_precision(reason="f32r softmax denom"):
                        nc.vector.reciprocal(rec[DK:DA, 0:QBLK], C0[DK:DA])
                        nc.vector.reciprocal(rec[DK:DA, QBLK:], C1[DK:DA])
                    rb0 = psM.tile([P, QBLK], F32, tag="misc", name="rb0")
                    rb1 = psM.tile([P, QBLK], F32, tag="misc", name="rb1")
                    nc.tensor.matmul(rb0[0:DK], ones_sb[DK:DA, 0:DK],
                                     rec[DK:DA, 0:QBLK], start=True, stop=True,
                                     tile_position=(64, 0))
                    nc.tensor.matmul(rb1[0:DK], ones_sb[DK:DA, 0:DK],
                                     rec[DK:DA, QBLK:], start=True, stop=True,
                                     tile_position=(64, 0))
                    rb_sb = small.tile([P, 2 * QBLK], F32, tag="rb_sb")
                    nc.vector.tensor_copy(rb_sb[0:DK, 0:QBLK], rb0[0:DK])
                    nc.vector.tensor_copy(rb_sb[0:DK, QBLK:], rb1[0:DK])
                    ctx0_sb = small.tile([P, QBLK], F32R, tag="ctx0_sb")
                    ctx1_sb = small.tile([P, QBLK], F32R, tag="ctx1_sb")
                    nc.vector.tensor_mul(ctx0_sb[0:DK], C0[0:DK],
                                         rb_sb[0:DK, 0:QBLK])
                    nc.vector.tensor_mul(ctx1_sb[0:DK], C1[0:DK],
                                         rb_sb[0:DK, QBLK:])
                    assert QBLK % TPC == 0
                    for sub in range(QBLK // TPC):
                        chunk = (b * T + qi * QBLK) // TPC + sub
                        nc.sync.dma_start(a2a_in[chunk, 0:DK],
                                          ctx0_sb[0:DK, ts(sub, TPC)])
                        nc.sync.dma_start(a2a_in[chunk, DK:P],
                                          ctx1_sb[0:DK, ts(sub, TPC)])

            # ---- phase 3: AllToAll over token slices --------------------
            nc.gpsimd.collective_compute(
                "AllToAll",
                mybir.AluOpType.bypass,
                replica_groups=[list(range(NCORES))],
                ins=[a2a_in[:].opt()],
                outs=[a2a_out[:].opt()],
            )

            # ---- phase 4: output projection (token-sharded) -------------
            ctxg = outg.tile([P, KO, TPC], F32R, tag="ctxg")
            nc.sync.dma_start(ctxg[:], a2a_out[:].rearrange("j p t -> p j t"))
            for tb in range(TB):
                o_sb = outp.tile([P, D], F32, tag="o_sb")
                for oh in range(D // QBLK):
                    po = psA.tile([P, QBLK], F32, tag="pp")
                    for ko in range(KO):
                        nc.tensor.matmul(po[:], ctxg[:, ko, ts(tb, P)],
                                         wo_sb[:, ko, ts(oh, QBLK)],
                                         start=(ko == 0), stop=(ko == KO - 1))
                    nc.vector.tensor_add(o_sb[:, ts(oh, QBLK)], po[:],
                                         bo_sb[:, ts(oh, QBLK)])
                nc.sync.dma_start(out_d.ap()[ts(tb, P), :], o_sb[:])

    nc.compile()
    return nc


_NC_CACHE = {}


def _get_nc(B, T):
    key = (B, T)
    if key not in _NC_CACHE:
        _NC_CACHE[key] = build_nc(B, T)
    return _NC_CACHE[key]


def make_in_maps(x, Wq, bq, Wk, bk, Wv, bv, Wo, bo):
    B, T, _ = x.shape
    NTOK = B * T
    x = np.asarray(x, np.float32)
    xT = np.ascontiguousarray(x.reshape(NTOK, D).T)
    woT = np.ascontiguousarray(np.asarray(Wo, np.float32).T)
    bo = np.asarray(bo, np.float32)
    # causal staircase: mask[kk, c] = 1 iff c >= kk + (QBLK - KBLK)
    mask = (np.arange(2 * QBLK - KBLK)[None, :]
            >= (np.arange(P)[:, None] + (QBLK - KBLK)))
    mask = mask.astype(np.float32)
    ident = np.eye(P, dtype=np.float32)
    ones = np.ones((P, P), np.float32)
    in_maps = []
    for c in range(NCORES):
        sl = slice(DSL * c, DSL * (c + 1))
        in_maps.append({
            "xT": xT,
            "wqT": np.ascontiguousarray(np.asarray(Wq, np.float32)[sl].T) * 0.125,
            "wkT": np.ascontiguousarray(np.asarray(Wk, np.float32)[sl].T),
            "wvT": np.ascontiguousarray(np.asarray(Wv, np.float32)[sl].T),
            "woT": woT,
            "bq": (np.asarray(bq, np.float32)[sl] * 0.125).reshape(DSL, 1),
            "bk": np.asarray(bk, np.float32)[sl].reshape(DSL, 1),
            "bv": np.asarray(bv, np.float32)[sl].reshape(DSL, 1),
            "bo": bo,
            "mask": mask,
            "ident": ident,
            "ones": ones,
        })
    return in_maps


LAST_RESULTS = None


def kernel(x, Wq, bq, Wk, bk, Wv, bv, Wo, bo, trace=False, trace_cores=None):
    global LAST_RESULTS
    B, T, _ = x.shape
    nc = _get_nc(B, T)
    in_maps = make_in_maps(x, Wq, bq, Wk, bk, Wv, bv, Wo, bo)
    kw = {}
    if trace:
        kw = dict(trace=True, trace_cores=trace_cores)
    res = bass_utils.run_bass_kernel_spmd(nc, in_maps,
                                          core_ids=list(range(NCORES)), **kw)
    LAST_RESULTS = res
    out = np.concatenate([res.results[c]["out"] for c in range(NCORES)], axis=0)
    return out.reshape(B, T, D)


# revision 56
# speedup vs baseline: 1.0289x; 1.0289x over previous
"""Multi-head attention (B=2, T=2048, D=1024, H=16, causal) on 8 TRN2 NeuronCores.

Sharding strategy (tensor-parallel over heads + token-parallel epilogue):
  - Each core c owns heads (2c, 2c+1)  -> a 128-wide slice of the QKV output dim.
  - QKV projections:  qT/kT/vT [128, B*T] computed from replicated x^T
    (feature-major) with host-pre-transposed weight slices.
  - Attention: flash-style streaming over 128-wide key blocks with
    transposed score tiles  S^T [k=128, q=512]; exp on ScalarE; causal
    staircase mask via a precomputed [128, 1024] 0/1 constant (DVE mult,
    diagonal tiles only); ctx^T and softmax denominators accumulate in PSUM
    (ones-column matmuls for the partition-dim sums).
  - AllToAll over the token dim redistributes ctx^T so core c holds
    ctx[tokens 512c:512(c+1), all 1024 cols], i.e. the minimal-traffic
    exchange (2 MB per core).
  - Output projection is token-sharded: each core computes its 512 output
    rows with the full (host-pre-transposed) Wo^T; host concatenates.

All matmuls run as float32r (1 cycle/row when the moving dim >= 256);
transposes stay float32 (exact).
"""

import numpy as np

import concourse.bacc as bacc
import concourse.bass as bass
import concourse.mybir as mybir
import concourse.tile as tile
from concourse import bass_utils
from concourse.bass import ts

D = 1024
H = 16
DK = D // H  # 64
NCORES = 8
HPC = H // NCORES  # heads per core = 2
DSL = HPC * DK  # per-core QKV output slice = 128
P = 128
QBLK = 512
KBLK = 128

F32 = mybir.dt.float32
F32R = mybir.dt.float32r


def build_nc(B=2, T=2048):
    """Build the SPMD Bass module (identical program on all 8 cores)."""
    NTOK = B * T
    TPC = NTOK // NCORES  # tokens per core for the output projection
    KO = D // P  # 8 contraction chunks for the projections
    NKB = T // KBLK  # key blocks per batch
    NQB = T // QBLK  # query blocks per batch
    TB = TPC // P  # 128-token sub-blocks in the output projection

    nc = bacc.Bacc("TRN2", target_bir_lowering=False, debug=False,
                   num_devices=NCORES)

    # ---- DRAM I/O ------------------------------------------------------
    xT_d = nc.dram_tensor("xT", [D, NTOK], F32R, kind="ExternalInput")
    wqT_d = nc.dram_tensor("wqT", [D, DSL], F32R, kind="ExternalInput")
    wkT_d = nc.dram_tensor("wkT", [D, DSL], F32R, kind="ExternalInput")
    wvT_d = nc.dram_tensor("wvT", [D, DSL], F32R, kind="ExternalInput")
    woT_d = nc.dram_tensor("woT", [D, D], F32R, kind="ExternalInput")
    bq_d = nc.dram_tensor("bq", [DSL, 1], F32, kind="ExternalInput")
    bk_d = nc.dram_tensor("bk", [DSL, 1], F32, kind="ExternalInput")
    bv_d = nc.dram_tensor("bv", [DSL, 1], F32, kind="ExternalInput")
    bo_d = nc.dram_tensor("bo", [D], F32, kind="ExternalInput")
    mask_d = nc.dram_tensor("mask", [P, 2 * QBLK - KBLK], F32R,
                            kind="ExternalInput")
    ident_d = nc.dram_tensor("ident", [P, P], F32R, kind="ExternalInput")
    ones_d = nc.dram_tensor("ones", [P, P], F32R, kind="ExternalInput")
    out_d = nc.dram_tensor("out", [TPC, D], F32, kind="ExternalOutput")

    with tile.TileContext(nc) as tc:
        with (
            tc.tile_pool(name="consts", bufs=1) as consts,
            tc.tile_pool(name="acts", bufs=1) as acts,
            tc.tile_pool(name="xin", bufs=2) as xin,
            tc.tile_pool(name="attn", bufs=3) as attn_pool,
            tc.tile_pool(name="small", bufs=1) as small,
            tc.tile_pool(name="outg", bufs=1) as outg,
            tc.tile_pool(name="outp", bufs=2) as outp,
            tc.tile_pool(name="psA", bufs=2, space="PSUM") as psA,
            tc.tile_pool(name="psC", bufs=2, space="PSUM") as psC,
            tc.tile_pool(name="dram", bufs=2, space="DRAM") as dram,
        ):
            # ---- constants / weights in SBUF ----------------------------
            wq_sb = consts.tile([P, KO, DSL], F32R, tag="wq")
            wk_sb = consts.tile([P, KO, DSL], F32R, tag="wk")
            wv_sb = consts.tile([P, KO, DSL], F32R, tag="wv")
            wo_sb = consts.tile([P, KO, D], F32R, tag="wo")
            nc.sync.dma_start(wq_sb[:], wqT_d.ap().rearrange("(ko p) m -> p ko m", p=P))
            nc.sync.dma_start(wk_sb[:], wkT_d.ap().rearrange("(ko p) m -> p ko m", p=P))
            nc.sync.dma_start(wv_sb[:], wvT_d.ap().rearrange("(ko p) m -> p ko m", p=P))
            nc.sync.dma_start(wo_sb[:], woT_d.ap().rearrange("(ko p) m -> p ko m", p=P))
            bq_sb = consts.tile([P, 1], F32, tag="bq")
            bk_sb = consts.tile([P, 1], F32, tag="bk")
            bv_sb = consts.tile([P, 1], F32, tag="bv")
            nc.sync.dma_start(bq_sb[:], bq_d.ap())
            nc.sync.dma_start(bk_sb[:], bk_d.ap())
            nc.sync.dma_start(bv_sb[:], bv_d.ap())
            bo_sb = consts.tile([P, D], F32, tag="bo")
            nc.sync.dma_start(bo_sb[:], bo_d.ap()[None, :].to_broadcast((P, D)))
            mask_sb = consts.tile([P, 2 * QBLK - KBLK], F32R, tag="mask")
            nc.sync.dma_start(mask_sb[:], mask_d.ap())
            ident_sb = consts.tile([P, P], F32R, tag="ident")
            nc.sync.dma_start(ident_sb[:], ident_d.ap())

            # ---- phase 1: QKV projections (feature-major outputs) -------
            # Token tiles processed in PAIRS sharing one stationary load per
            # (proj, ko); one wide [P, 2*QBLK] PSUM + one wide epilogue ACT.
            qT = acts.tile([P, NTOK], F32R, tag="qT")
            kT = acts.tile([P, NTOK], F32R, tag="kT")
            vT = acts.tile([P, NTOK], F32R, tag="vT")
            xT_r = xT_d.ap().rearrange("(ko p) t -> p ko t", p=P)
            n_tpair = NTOK // (2 * QBLK)
            for i in range(n_tpair):
                xt0 = xin.tile([P, KO, QBLK], F32R, tag="xt")
                xt1 = xin.tile([P, KO, QBLK], F32R, tag="xt")
                nc.sync.dma_start(xt0[:], xT_r[:, :, ts(2 * i, QBLK)])
                nc.sync.dma_start(xt1[:], xT_r[:, :, ts(2 * i + 1, QBLK)])
                for w_sb, b_sb, dst in ((wq_sb, bq_sb, qT),
                                        (wk_sb, bk_sb, kT),
                                        (wv_sb, bv_sb, vT)):
                    ps = psA.tile([P, 2 * QBLK], F32, tag="sp")
                    for ko in range(KO):
                        nc.tensor.matmul(ps[:, 0:QBLK], w_sb[:, ko], xt0[:, ko],
                                         start=(ko == 0), stop=(ko == KO - 1))
                        nc.tensor.matmul(ps[:, QBLK:], w_sb[:, ko], xt1[:, ko],
                                         start=(ko == 0), stop=(ko == KO - 1))
                    nc.scalar.activation(dst[:, ts(i, 2 * QBLK)], ps[:],
                                         mybir.ActivationFunctionType.Identity,
                                         bias=b_sb[:, 0:1])

            # ---- phase 1.5: v -> natural layout [ktok, d|1] -------------
            # per key-block: [v_h0 (64) | ones (1) | v_h1 (64) | ones (1)]
            # so the ctx matmul's 65th output row is the softmax denominator.
            DA = DK + 1  # 65
            v_nat = acts.tile([P, NTOK // P, 2 * DA], F32R, tag="v_nat")
            nc.sync.dma_start(v_nat[:, :, DK], ones_d.ap()[:, 0:NTOK // P])
            nc.sync.dma_start(v_nat[:, :, DA + DK], ones_d.ap()[:, 0:NTOK // P])
            for j in range(NTOK // P):
                ptf = psA.tile([P, 2 * QBLK], F32R, tag="sp", name="ptf")
                pt = ptf[:, :P]
                nc.tensor.transpose(pt[:], vT[:, ts(j, P)], ident_sb[:])
                nc.vector.tensor_copy(v_nat[:, j, 0:DK], pt[:, 0:DK])
                nc.vector.tensor_copy(v_nat[:, j, DA:DA + DK], pt[:, DK:P])

            # ---- phase 2: causal attention (2 heads, feature-major) -----
            a2a_in = dram.tile([NCORES, P, TPC], F32R, tag="a2a_in")
            a2a_out = dram.tile([NCORES, P, TPC], F32R, tag="a2a_out")
            for b in range(B):
                for qi in range(NQB):
                    q_sl = ts(b * T // QBLK + qi, QBLK)  # global token slice
                    nkb = (qi + 1) * (QBLK // KBLK)
                    C0 = psC.tile([P, QBLK], F32, tag="ctx0")
                    C1 = psC.tile([P, QBLK], F32, tag="ctx1")

                    def emit_ctx(pend):
                        ap_, jjp, st, sp = pend
                        nc.tensor.matmul(C0[0:DA], v_nat[:, jjp, 0:DA],
                                         ap_[:, 0:QBLK], start=st, stop=sp)
                        nc.tensor.matmul(C1[0:DA], v_nat[:, jjp, DA:2 * DA],
                                         ap_[:, QBLK:], start=st, stop=sp)

                    pend = None
                    for ki in range(nkb):
                        k_sl = ts(b * T // KBLK + ki, KBLK)
                        jj = b * NKB + ki
                        doff = ki * KBLK - qi * QBLK
                        diag = doff >= 0
                        sp_t = psA.tile([P, 2 * QBLK], F32, tag="sp")
                        nc.tensor.matmul(sp_t[:, 0:QBLK],
                                         kT[0:DK, k_sl], qT[0:DK, q_sl],
                                         start=True, stop=not diag,
                                         tile_position=(0, 0))
                        nc.tensor.matmul(sp_t[:, QBLK:],
                                         kT[DK:P, k_sl], qT[DK:P, q_sl],
                                         start=True, stop=not diag,
                                         tile_position=(64, 0))
                        if diag:
                            # causal mask: accumulate a -300 staircase bias
                            # (identity-stationary matmul); exp gives 0.
                            s = QBLK - KBLK - doff
                            m = mask_sb[:, s:s + QBLK]
                            nc.tensor.matmul(sp_t[:, 0:QBLK], ident_sb[:], m,
                                             start=False, stop=True)
                            nc.tensor.matmul(sp_t[:, QBLK:], ident_sb[:], m,
                                             start=False, stop=True)
                        a_p = attn_pool.tile([P, 2 * QBLK], F32R, tag="ap")
                        nc.scalar.activation(a_p[:], sp_t[:],
                                             mybir.ActivationFunctionType.Exp)
                        # software pipeline: issue ctx of the PREVIOUS k-block
                        # after this block's scores, so PE runs ahead of ACT.
                        if pend is not None:
                            emit_ctx(pend)
                        pend = (a_p, jj, ki == 0, ki == nkb - 1)
                    emit_ctx(pend)
                    # normalize: ctx^T * (1/sum); sums sit in row 64 (the
                    # ones column of v_aug).  approx-recip, then partition
                    # broadcast via a tiny SBUF->SBUF DMA.
                    rec = small.tile([P, 2 * QBLK], F32, tag="rec")
                    nc.vector.reciprocal(rec[DK:DA, 0:QBLK], C0[DK:DA])
                    nc.vector.reciprocal(rec[DK:DA, QBLK:], C1[DK:DA])
                    rec_dr = dram.tile([1, 2 * QBLK], F32, tag="rec_dr")
                    nc.sync.dma_start(rec_dr[:], rec[DK:DA, :])
                    rb_sb = small.tile([P, 2 * QBLK], F32, tag="rb_sb")
                    nc.sync.dma_start(rb_sb[0:DK, :],
                                      rec_dr[:].to_broadcast((DK, 2 * QBLK)))
                    ctx0_sb = small.tile([P, QBLK], F32R, tag="ctx0_sb")
                    ctx1_sb = small.tile([P, QBLK], F32R, tag="ctx1_sb")
                    nc.vector.tensor_mul(ctx0_sb[0:DK], C0[0:DK],
                                         rb_sb[0:DK, 0:QBLK])
                    nc.vector.tensor_mul(ctx1_sb[0:DK], C1[0:DK],
                                         rb_sb[0:DK, QBLK:])
                    assert QBLK % TPC == 0
                    for sub in range(QBLK // TPC):
                        chunk = (b * T + qi * QBLK) // TPC + sub
                        nc.sync.dma_start(a2a_in[chunk, 0:DK],
                                          ctx0_sb[0:DK, ts(sub, TPC)])
                        nc.sync.dma_start(a2a_in[chunk, DK:P],
                                          ctx1_sb[0:DK, ts(sub, TPC)])

            # ---- phase 3: AllToAll over token slices --------------------
            nc.gpsimd.collective_compute(
                "AllToAll",
                mybir.AluOpType.bypass,
                replica_groups=[list(range(NCORES))],
                ins=[a2a_in[:].opt()],
                outs=[a2a_out[:].opt()],
            )

            # ---- phase 4: output projection (token-sharded) -------------
            ctxg = outg.tile([P, KO, TPC], F32R, tag="ctxg")
            nc.sync.dma_start(ctxg[:], a2a_out[:].rearrange("j p t -> p j t"))
            for tb in range(TB):
                po = psA.tile([P, 2 * QBLK], F32, tag="sp")
                for ko in range(KO):
                    nc.tensor.matmul(po[:, 0:QBLK], ctxg[:, ko, ts(tb, P)],
                                     wo_sb[:, ko, 0:QBLK],
                                     start=(ko == 0), stop=(ko == KO - 1))
                    nc.tensor.matmul(po[:, QBLK:], ctxg[:, ko, ts(tb, P)],
                                     wo_sb[:, ko, QBLK:],
                                     start=(ko == 0), stop=(ko == KO - 1))
                o_sb = outp.tile([P, D], F32, tag="o_sb")
                nc.vector.tensor_add(o_sb[:], po[:], bo_sb[:])
                nc.sync.dma_start(out_d.ap()[ts(tb, P), :], o_sb[:])

    nc.compile()
    return nc


_NC_CACHE = {}


def _get_nc(B, T):
    key = (B, T)
    if key not in _NC_CACHE:
        _NC_CACHE[key] = build_nc(B, T)
    return _NC_CACHE[key]


def make_in_maps(x, Wq, bq, Wk, bk, Wv, bv, Wo, bo):
    B, T, _ = x.shape
    NTOK = B * T
    x = np.asarray(x, np.float32)
    xT = np.ascontiguousarray(x.reshape(NTOK, D).T)
    woT = np.ascontiguousarray(np.asarray(Wo, np.float32).T)
    bo = np.asarray(bo, np.float32)
    # causal staircase bias: 0 where allowed (c >= kk + (QBLK-KBLK)),
    # -300 where masked; accumulated into scores via an identity-stationary
    # matmul so exp() of masked entries underflows to zero.
    keep = (np.arange(2 * QBLK - KBLK)[None, :]
            >= (np.arange(P)[:, None] + (QBLK - KBLK)))
    mask = np.where(keep, 0.0, -300.0).astype(np.float32)
    ident = np.eye(P, dtype=np.float32)
    ones = np.ones((P, P), np.float32)
    in_maps = []
    for c in range(NCORES):
        sl = slice(DSL * c, DSL * (c + 1))
        in_maps.append({
            "xT": xT,
            "wqT": np.ascontiguousarray(np.asarray(Wq, np.float32)[sl].T) * 0.125,
            "wkT": np.ascontiguousarray(np.asarray(Wk, np.float32)[sl].T),
            "wvT": np.ascontiguousarray(np.asarray(Wv, np.float32)[sl].T),
            "woT": woT,
            "bq": (np.asarray(bq, np.float32)[sl] * 0.125).reshape(DSL, 1),
            "bk": np.asarray(bk, np.float32)[sl].reshape(DSL, 1),
            "bv": np.asarray(bv, np.float32)[sl].reshape(DSL, 1),
            "bo": bo,
            "mask": mask,
            "ident": ident,
            "ones": ones,
        })
    return in_maps


LAST_RESULTS = None


def kernel(x, Wq, bq, Wk, bk, Wv, bv, Wo, bo, trace=False, trace_cores=None):
    global LAST_RESULTS
    B, T, _ = x.shape
    nc = _get_nc(B, T)
    in_maps = make_in_maps(x, Wq, bq, Wk, bk, Wv, bv, Wo, bo)
    kw = {}
    if trace:
        kw = dict(trace=True, trace_cores=trace_cores)
    res = bass_utils.run_bass_kernel_spmd(nc, in_maps,
                                          core_ids=list(range(NCORES)), **kw)
    LAST_RESULTS = res
    out = np.concatenate([res.results[c]["out"] for c in range(NCORES)], axis=0)
    return out.reshape(B, T, D)
